# revision 34
# baseline (speedup 1.0000x reference)
"""Expert-parallel MoE (top-1 routing) kernel for 8 TRN2 NeuronCores.

Strategy (per the expert-parallel sharding hint): the 8 experts are sharded
1:1 across the 8 cores. The router is a 0.1%-of-FLOPs linear; it is computed
host-side in float64 to decide the token->expert dispatch (the all-to-all is
realized as the host->device sharding itself: each token's activations are
DMA'd only to the core owning its expert). Each core then runs the dense
expert MLP  y = (silu(x @ gw.T) * (x @ up.T)) @ dw.T  over its gathered
tokens (padded to a uniform capacity C) with fp32 PSUM accumulation.

Layout: everything on device is kept "activation-transposed" so all three
matmuls contract over the partition dimension with zero on-device transposes:
  g_T[i_tile] = sum_k gwT[k, i].T @ x_T[k]      (psum [128(I), C])
  a_T = silu(g_T) * u_T                          (sbuf bf16)
  y_T[m_tile] += dwT[i, m].T @ a_T[i]            (psum [128(H), C], 22-step acc)

Precision: gate/up weights are stored as fp8-e3m4 (power-of-two pre-scale,
descale folded exactly into the silu scale and the DVE multiply), halving
their HBM traffic; down weights and activations stay bf16. The PE runs
mixed-dtype matmuls (fp8 stationary, bf16 moving) at the bf16 rate.

DMA: one SP HWDGE queue carrying x, then gate/up + down weight chunks in
PE-consumption order. Chunk sizes ramp [1,1,2,2,4,4,4,4] i-tiles: small
first chunks let the PE start ~5us earlier; later 8KB-per-partition
descriptors amortize the ~120ns fixed per-descriptor DMA cost (measured:
4KB descs -> ~330 GB/s, 6KB -> ~410 GB/s aggregate).

Software pipeline (raw bass, per-engine streams): the down-projection
matmuls for i-tile i-1 are issued AFTER gate/up of i-tile i, so the PE never
stalls waiting for the ACT->DVE chain of the same iteration:
  SP     : x DMA, interleaved w8/wd chunk DMAs, y DMA (cols 0..5C)
  PE     : per i: 8 g-matmuls, 8 u-matmuls, then 8 y-matmuls of i-1
  ACT    : per i: silu(g)->sbuf (with 1/Sg descale); tail: 3 psum->sbuf
           copies + y DMA (cols 5C..8C) on its own HWDGE queue
  DVE    : per i: a_T[i] = silu_g * u' * (1/Su) (bf16); tail: 5 psum->sbuf
"""

import math

import numpy as np
import ml_dtypes
from contextlib import ExitStack

import concourse.bass as bass
import concourse.mybir as mybir
from concourse.alu_op_type import AluOpType
from concourse.bass_utils import run_bass_kernel_spmd

S, B, H, I, E = 512, 2, 1024, 2816, 8
KT, IT, MT = H // 128, I // 128, H // 128  # 8, 22, 8
_BF = mybir.dt.bfloat16
_F8 = mybir.dt.float8e3  # e3m4
_F32 = mybir.dt.float32

GU_FP8 = True  # gate/up weights in fp8-e3m4 (halves their HBM bytes)

# CoreSim-only: gate the PE warm-up matmuls on a memset of their input so
# the simulator's uninitialized-read checker stays quiet. On hardware the
# warm-up reads garbage SBUF on purpose (results are discarded), and waiting
# would delay the clock ramp.
SIM_WARMUP_WAIT = False

# i-tiles per weight-DMA chunk (sums to IT=22): small first chunks so the
# in-order stream stays ahead of the PE during warm-up, then 3-tile chunks
# whose 6KB-per-partition descriptors hit the DMA engines' per-descriptor
# sweet spot (~200ns/desc up to ~6KB payload; 8KB costs two quanta).
GROUPS = [1, 1, 1, 2, 2, 3, 3, 3, 3, 3]
GSTART = [sum(GROUPS[:g]) for g in range(len(GROUPS))]
NG = len(GROUPS)
assert sum(GROUPS) == IT

_nc_cache: dict = {}


def _build(C: int, inv_sg: float, inv_su: float) -> bass.Bass:
    """One-core program; SPMD across 8 cores (same shapes, per-core data)."""
    nc = bass.Bass()
    GUW = 2 * KT * 128  # gate|up cols per i-tile (2048)
    DW = MT * 128  # down cols per i-tile (1024)
    xt = nc.dram_tensor("xt", [128, KT * C], _BF, kind="ExternalInput")
    if GU_FP8:
        w8t = nc.dram_tensor("w8t", [128, IT * GUW], _F8, kind="ExternalInput")
        wdt = nc.dram_tensor("wdt", [128, IT * DW], _BF, kind="ExternalInput")
    else:
        wt = nc.dram_tensor("wt", [128, IT * (GUW + DW)], _BF, kind="ExternalInput")
    yt = nc.dram_tensor("yt", [128, MT * C], _BF, kind="ExternalOutput")

    assert C + 256 <= 512, "two y slices must fit one PSUM bank"

    with ExitStack() as ctx:
        x_sb = ctx.enter_context(nc.sbuf_tensor([128, KT * C], _BF))
        if GU_FP8:
            w8_sb = ctx.enter_context(nc.sbuf_tensor([128, IT * GUW], _F8))
            wd_sb = ctx.enter_context(nc.sbuf_tensor([128, IT * DW], _BF))
        else:
            w_sb = ctx.enter_context(nc.sbuf_tensor([128, IT * (GUW + DW)], _BF))
        sg_sb = ctx.enter_context(nc.sbuf_tensor([128, IT * C], _F32))
        a_sb = ctx.enter_context(nc.sbuf_tensor([128, IT * C], _BF))
        # y writeback in bf16: halves the tail DMA and doubles copy rate
        # (costs ~0.2% extra output quantization, well inside the budget)
        y_sb = ctx.enter_context(nc.sbuf_tensor([128, MT * C], _BF))
        # every PSUM tensor is one full 2 KiB bank ([128, 512] f32): matmul
        # outputs must not cross bank boundaries, and the bump allocator
        # would otherwise pack tensors across banks
        g_ps = [
            ctx.enter_context(nc.psum_tensor(f"g_ps{j}", [128, 512], _F32))
            for j in range(2)
        ]
        u_ps = [
            ctx.enter_context(nc.psum_tensor(f"u_ps{j}", [128, 512], _F32))
            for j in range(2)
        ]
        y_ps = [
            ctx.enter_context(nc.psum_tensor(f"y_ps{j}", [128, 512], _F32))
            for j in range(4)
        ]

        def yslice(m):
            return y_ps[m // 2][:, (m % 2) * 256 : (m % 2) * 256 + C]

        def gw_tile(i, k):
            if GU_FP8:
                base = i * GUW
                return w8_sb[:, base + k * 128 : base + (k + 1) * 128]
            base = i * (GUW + DW)
            return w_sb[:, base + k * 128 : base + (k + 1) * 128]

        def uw_tile(i, k):
            if GU_FP8:
                base = i * GUW + KT * 128
                return w8_sb[:, base + k * 128 : base + (k + 1) * 128]
            base = i * (GUW + DW) + KT * 128
            return w_sb[:, base + k * 128 : base + (k + 1) * 128]

        def dw_tile(i, m):
            if GU_FP8:
                base = i * DW
                return wd_sb[:, base + m * 128 : base + (m + 1) * 128]
            base = i * (GUW + DW) + GUW
            return w_sb[:, base + m * 128 : base + (m + 1) * 128]

        warm_sb = ctx.enter_context(nc.sbuf_tensor([128, 512], _BF))

        x_sem = ctx.enter_context(nc.semaphore())
        x2_sem = ctx.enter_context(nc.semaphore(name="x2_sem"))
        warm_sem = ctx.enter_context(nc.semaphore(name="warm_sem"))
        w0_sem = [
            ctx.enter_context(nc.semaphore(name=f"w0_sem{j}")) for j in range(4)
        ]
        w_sem = [ctx.enter_context(nc.semaphore(name=f"w_sem{g}")) for g in range(NG)]
        if GU_FP8:
            wd_sem = [
                ctx.enter_context(nc.semaphore(name=f"wd_sem{g}")) for g in range(NG)
            ]
        pe_g = ctx.enter_context(nc.semaphore())
        pe_u = ctx.enter_context(nc.semaphore())
        pe_done = ctx.enter_context(nc.semaphore())
        act_sem = ctx.enter_context(nc.semaphore())
        dve_sem = ctx.enter_context(nc.semaphore())
        dma_sem = ctx.enter_context(nc.semaphore())

        # y writeback split: DVE copies m=0..4 (SP DMA), ACT copies m=5..7
        # (ACT's own HWDGE queue) — balances the two copy engines.
        MSPLIT = 5

        block = ctx.enter_context(nc.Block())

        @block.sync
        def _(sync):
            # one SP HWDGE queue, chunks in exact PE-consumption order
            # (x split in two around the first gate/up chunk; down chunks
            # staggered one group behind gate/up since their first use is
            # one i-tile later). The queue's in-order descriptor stream
            # sustains ~410+ GB/s at 6KB/desc.
            def w8_dma(g):
                i0, gn = GSTART[g], GROUPS[g]
                nc.sync.dma_start(
                    w8_sb[:, i0 * GUW : (i0 + gn) * GUW],
                    w8t[:, i0 * GUW : (i0 + gn) * GUW],
                ).then_inc(w_sem[g], 16)

            def wd_dma(g):
                i0, gn = GSTART[g], GROUPS[g]
                nc.sync.dma_start(
                    wd_sb[:, i0 * DW : (i0 + gn) * DW],
                    wdt[:, i0 * DW : (i0 + gn) * DW],
                ).then_inc(wd_sem[g], 16)

            XH = KT * C // 2
            if GU_FP8:
                # i-tile 0's gate/up ship as four 64KB sub-chunks threaded
                # between the x halves, so the PE's first matmul can start
                # after just ~0.25MB has landed.
                SUB = GUW // 4  # 512 cols = 4 k-tiles of one projection
                nc.sync.dma_start(x_sb[:, :XH], xt[:, :XH]).then_inc(x_sem, 16)
                for j in range(2):
                    nc.sync.dma_start(
                        w8_sb[:, j * SUB : (j + 1) * SUB],
                        w8t[:, j * SUB : (j + 1) * SUB],
                    ).then_inc(w0_sem[j], 16)
                nc.sync.dma_start(x_sb[:, XH:], xt[:, XH:]).then_inc(x2_sem, 16)
                for j in range(2, 4):
                    nc.sync.dma_start(
                        w8_sb[:, j * SUB : (j + 1) * SUB],
                        w8t[:, j * SUB : (j + 1) * SUB],
                    ).then_inc(w0_sem[j], 16)
                for g in range(1, NG):
                    w8_dma(g)
                    wd_dma(g - 1)
                wd_dma(NG - 1)
            else:
                nc.sync.dma_start(x_sb[:], xt[:]).then_inc(x_sem, 16)
                for g in range(NG):
                    i0, gn = GSTART[g], GROUPS[g]
                    W = GUW + DW
                    nc.sync.dma_start(
                        w_sb[:, i0 * W : (i0 + gn) * W],
                        wt[:, i0 * W : (i0 + gn) * W],
                    ).then_inc(w_sem[g], 16)
            nc.sync.wait_ge(dve_sem, IT + MSPLIT)
            nc.sync.dma_start(
                yt[:, : MSPLIT * C], y_sb[:, : MSPLIT * C]
            ).then_inc(dma_sem, 16)
            nc.sync.wait_ge(dma_sem, 32)

        def y_block(i, stop, inc_each=False):
            for m in range(MT):
                # start=True clears has_written for the WHOLE psum bank,
                # so only the first (even) slice of each bank may set it;
                # the odd slice's first write then lands on cleared
                # has_written and overwrites cleanly.
                mm = nc.tensor.matmul(
                    yslice(m),
                    dw_tile(i, m),
                    a_sb[:, i * C : (i + 1) * C],
                    start=(i == 0 and m % 2 == 0),
                    stop=stop,
                    skip_group_check=True,
                )
                if inc_each:
                    mm.then_inc(pe_done, 1)

        group_of = [g for g in range(NG) for _ in range(GROUPS[g])]

        @block.tensor
        def _(tensor):
            if GU_FP8:
                # warm-up: garbage matmuls ramp the PE clock to full while
                # the first weight chunks stream in (~3.4us: 1 low-clock +
                # 5 mid-clock + 2 full); psum bank 0 is re-initialized
                # (start=True) by the first real matmul.
                if SIM_WARMUP_WAIT:
                    nc.tensor.wait_ge(warm_sem, 1)
                for _ in range(8):
                    nc.tensor.matmul(
                        g_ps[0][:],
                        warm_sb[:, :128],
                        warm_sb[:],
                        start=True,
                        stop=True,
                    )
            nc.tensor.wait_ge(x_sem, 16)
            g = -1
            for i in range(IT):
                pp = i % 2
                if g + 1 < NG and i == GSTART[g + 1]:
                    g += 1
                    if not (GU_FP8 and g == 0):
                        nc.tensor.wait_ge(w_sem[g], 16)
                if i >= 2:
                    # one wait covers both: a_T[i-2] ready for the y-block
                    # below, and g/u psum banks pp free (mul(i-2) drained)
                    nc.tensor.wait_ge(dve_sem, i - 1)
                for k in range(KT):
                    if GU_FP8 and i == 0 and k == 0:
                        nc.tensor.wait_ge(w0_sem[0], 16)
                    if GU_FP8 and i == 0 and k == KT // 2:
                        nc.tensor.wait_ge(x2_sem, 16)  # second x half landed
                        nc.tensor.wait_ge(w0_sem[1], 16)
                    mm = nc.tensor.matmul(
                        g_ps[pp][:, :C],
                        gw_tile(i, k),
                        x_sb[:, k * C : (k + 1) * C],
                        start=(k == 0),
                        stop=(k == KT - 1),
                    )
                mm.then_inc(pe_g, 1)
                for k in range(KT):
                    if GU_FP8 and i == 0 and k == 0:
                        nc.tensor.wait_ge(w0_sem[2], 16)
                    if GU_FP8 and i == 0 and k == KT // 2:
                        nc.tensor.wait_ge(w0_sem[3], 16)
                    mm = nc.tensor.matmul(
                        u_ps[pp][:, :C],
                        uw_tile(i, k),
                        x_sb[:, k * C : (k + 1) * C],
                        start=(k == 0),
                        stop=(k == KT - 1),
                    )
                mm.then_inc(pe_u, 1)
                if i >= 2:
                    # y-matmuls run TWO iterations behind: a_T[i-2] was
                    # finished ~2.4us ago, so ACT->DVE chain jitter never
                    # stalls the PE
                    if GU_FP8 and (i - 2) in GSTART:
                        # down weights gate only their first y use
                        nc.tensor.wait_ge(wd_sem[group_of[i - 2]], 16)
                    y_block(i - 2, stop=False)
            nc.tensor.wait_ge(dve_sem, IT - 1)
            if GU_FP8 and (IT - 2) in GSTART:
                nc.tensor.wait_ge(wd_sem[group_of[IT - 2]], 16)
            y_block(IT - 2, stop=False)
            nc.tensor.wait_ge(dve_sem, IT)
            y_block(IT - 1, stop=True, inc_each=True)

        @block.scalar
        def _(scalar):
            for i in range(IT):
                pp = i % 2
                nc.scalar.wait_ge(pe_g, i + 1)
                nc.scalar.activation(
                    sg_sb[:, i * C : (i + 1) * C],
                    g_ps[pp][:, :C],
                    mybir.ActivationFunctionType.Silu,
                    scale=inv_sg,
                ).then_inc(act_sem, 1)
            for m in range(MSPLIT, MT):
                nc.scalar.wait_ge(pe_done, m + 1)
                nc.scalar.copy(y_sb[:, m * C : (m + 1) * C], yslice(m)).then_inc(
                    act_sem, 1
                )
            nc.scalar.wait_ge(act_sem, IT + (MT - MSPLIT))
            nc.scalar.dma_start(
                yt[:, MSPLIT * C :], y_sb[:, MSPLIT * C :]
            ).then_inc(dma_sem, 16)

        @block.vector
        def _(vector):
            if GU_FP8 and SIM_WARMUP_WAIT:
                nc.vector.memset(warm_sb[:], 0.0).then_inc(warm_sem, 1)
            for i in range(IT):
                pp = i % 2
                nc.vector.wait_ge(act_sem, i + 1)
                nc.vector.wait_ge(pe_u, i + 1)
                if GU_FP8:
                    # a = (u' * 1/Su) * silu_g   (1/Su is a power of two)
                    nc.vector.scalar_tensor_tensor(
                        a_sb[:, i * C : (i + 1) * C],
                        u_ps[pp][:, :C],
                        inv_su,
                        sg_sb[:, i * C : (i + 1) * C],
                        AluOpType.mult,
                        AluOpType.mult,
                    ).then_inc(dve_sem, 1)
                else:
                    nc.vector.tensor_mul(
                        a_sb[:, i * C : (i + 1) * C],
                        sg_sb[:, i * C : (i + 1) * C],
                        u_ps[pp][:, :C],
                    ).then_inc(dve_sem, 1)
            for m in range(MSPLIT):
                nc.vector.wait_ge(pe_done, m + 1)
                nc.vector.tensor_copy(
                    y_sb[:, m * C : (m + 1) * C], yslice(m)
                ).then_inc(dve_sem, 1)

    return nc


def _bf(x):
    return np.ascontiguousarray(x).astype(ml_dtypes.bfloat16)


def _pow2_scale(absmax: float, dt) -> float:
    fmax = float(ml_dtypes.finfo(dt).max)
    return 2.0 ** math.floor(math.log2((fmax * 0.5) / absmax))


def run(hidden_states, router_w, gate_w, up_w, down_w, trace=False):
    h = np.asarray(hidden_states, dtype=np.float32)
    rw = np.asarray(router_w, dtype=np.float32)
    gw = np.asarray(gate_w, dtype=np.float32)
    uw = np.asarray(up_w, dtype=np.float32)
    dw = np.asarray(down_w, dtype=np.float32)

    T = S * B
    hf = h.reshape(T, H)
    logits = hf.astype(np.float64) @ rw.astype(np.float64).T
    ids = logits.argmax(-1)
    idx = [np.where(ids == e)[0] for e in range(E)]
    maxc = max(len(s) for s in idx)
    C = max(128, -(-maxc // 4) * 4)

    if GU_FP8:
        sg = _pow2_scale(float(np.abs(gw).max()), ml_dtypes.float8_e3m4)
        su = _pow2_scale(float(np.abs(uw).max()), ml_dtypes.float8_e3m4)
    else:
        sg = su = 1.0

    key = (C, sg, su)
    if key not in _nc_cache:
        _nc_cache[key] = _build(C, 1.0 / sg, 1.0 / su)
    nc = _nc_cache[key]

    in_maps = []
    for e in range(E):
        sel = idx[e]
        xp = np.zeros((C, H), np.float32)
        xp[: len(sel)] = hf[sel]
        # xt[p, k*C+c] = x[c, k*128+p]
        xt = _bf(xp.reshape(C, KT, 128).transpose(2, 1, 0).reshape(128, KT * C))
        # gwt[i, p, k*128+m] = gate_w[e][i*128+m, k*128+p]
        gwt = gw[e].reshape(IT, 128, KT, 128).transpose(0, 3, 2, 1).reshape(IT, 128, KT * 128)
        uwt = uw[e].reshape(IT, 128, KT, 128).transpose(0, 3, 2, 1).reshape(IT, 128, KT * 128)
        # dwt[i, p, m*128+mm] = down_w[e][m*128+mm, i*128+p]
        dwt = dw[e].reshape(MT, 128, IT, 128).transpose(2, 3, 0, 1).reshape(IT, 128, MT * 128)
        if GU_FP8:
            gu = np.concatenate([gwt * sg, uwt * su], axis=2)  # [IT,128,2048]
            w8 = np.ascontiguousarray(
                gu.transpose(1, 0, 2).reshape(128, IT * 2 * KT * 128)
            ).astype(ml_dtypes.float8_e3m4)
            wdv = _bf(dwt.transpose(1, 0, 2).reshape(128, IT * MT * 128))
            in_maps.append({"xt": xt, "w8t": w8, "wdt": wdv})
        else:
            wtv = _bf(
                np.concatenate([gwt, uwt, dwt], axis=2)
                .transpose(1, 0, 2)
                .reshape(128, IT * (2 * KT + MT) * 128)
            )
            in_maps.append({"xt": xt, "wt": wtv})

    res = run_bass_kernel_spmd(nc, in_maps, core_ids=list(range(E)), trace=trace)

    out = np.zeros((T, H), np.float32)
    for e in range(E):
        ytv = np.asarray(res.results[e]["yt"]).astype(np.float32)
        # y[c, m*128+p] = yt[p, m*C+c]
        y = ytv.reshape(128, MT, C).transpose(2, 1, 0).reshape(C, H)
        out[idx[e]] = y[: len(idx[e])]
    return out.reshape(S, B, H), res


def kernel(**inputs) -> np.ndarray:
    out, _ = run(**inputs)
    return out


# revision 48
# speedup vs baseline: 1.1114x; 1.1114x over previous
"""Expert-parallel MoE (top-1 routing) kernel for 8 TRN2 NeuronCores.

Strategy (per the expert-parallel sharding hint): the 8 experts are sharded
1:1 across the 8 cores. The router is a 0.1%-of-FLOPs linear; it is computed
host-side in float64 to decide the token->expert dispatch (the all-to-all is
realized as the host->device sharding itself: each token's activations are
DMA'd only to the core owning its expert). Each core then runs the dense
expert MLP  y = (silu(x @ gw.T) * (x @ up.T)) @ dw.T  over its gathered
tokens (padded to a uniform capacity C) with fp32 PSUM accumulation.

Layout: everything on device is kept "activation-transposed" so all three
matmuls contract over the partition dimension with zero on-device transposes:
  g_T[i_tile] = sum_k gwT[k, i].T @ x_T[k]      (psum [128(I), C])
  a_T = silu(g_T) * u_T                          (sbuf bf16)
  y_T[m_tile] += dwT[i, m].T @ a_T[i]            (psum [128(H), C], 22-step acc)

Precision: gate/up weights are stored as fp8-e3m4 (power-of-two pre-scale,
descale folded exactly into the silu scale and the DVE multiply), halving
their HBM traffic; down weights and activations stay bf16. The PE runs
mixed-dtype matmuls (fp8 stationary, bf16 moving) at the bf16 rate.

DMA: one SP HWDGE queue carrying x, then gate/up + down weight chunks in
PE-consumption order. Chunk sizes ramp [1,1,2,2,4,4,4,4] i-tiles: small
first chunks let the PE start ~5us earlier; later 8KB-per-partition
descriptors amortize the ~120ns fixed per-descriptor DMA cost (measured:
4KB descs -> ~330 GB/s, 6KB -> ~410 GB/s aggregate).

Software pipeline (raw bass, per-engine streams): the down-projection
matmuls for i-tile i-1 are issued AFTER gate/up of i-tile i, so the PE never
stalls waiting for the ACT->DVE chain of the same iteration:
  SP     : x DMA, interleaved w8/wd chunk DMAs, y DMA (cols 0..5C)
  PE     : per i: 8 g-matmuls, 8 u-matmuls, then 8 y-matmuls of i-1
  ACT    : per i: silu(g)->sbuf (with 1/Sg descale); tail: 3 psum->sbuf
           copies + y DMA (cols 5C..8C) on its own HWDGE queue
  DVE    : per i: a_T[i] = silu_g * u' * (1/Su) (bf16); tail: 5 psum->sbuf
"""

import math

import numpy as np
import ml_dtypes
from contextlib import ExitStack

import concourse.bass as bass
import concourse.mybir as mybir
from concourse.alu_op_type import AluOpType
from concourse.bass_utils import run_bass_kernel_spmd

S, B, H, I, E = 512, 2, 1024, 2816, 8
KT, IT, MT = H // 128, I // 128, H // 128  # 8, 22, 8
_BF = mybir.dt.bfloat16
_F8 = mybir.dt.float8e3  # e3m4
_F32 = mybir.dt.float32

GU_FP8 = True  # gate/up weights in fp8-e3m4 (halves their HBM bytes)

# CoreSim-only: gate the PE warm-up matmuls on a memset of their input so
# the simulator's uninitialized-read checker stays quiet. On hardware the
# warm-up reads garbage SBUF on purpose (results are discarded), and waiting
# would delay the clock ramp.
SIM_WARMUP_WAIT = False

Y_LAG = 2  # how many i-tiles the down-projection matmuls trail gate/up
WARMUP = 0  # PE clock warm-up matmuls (A/B-benched neutral; disabled)
NO_GPSIMD_DRAIN = True  # skip the idle GpSimd engine's costly exit drain

# i-tiles per weight-DMA chunk (sums to IT=22): small first chunks so the
# in-order stream stays ahead of the PE during warm-up, then 3-tile chunks
# whose 6KB-per-partition descriptors hit the DMA engines' per-descriptor
# sweet spot (~200ns/desc up to ~6KB payload; 8KB costs two quanta).
GROUPS = [1, 1, 1, 2, 2, 3, 3, 3, 3, 3]
GSTART = [sum(GROUPS[:g]) for g in range(len(GROUPS))]
NG = len(GROUPS)
assert sum(GROUPS) == IT


def GROUPS_KEY():
    return tuple(GROUPS)


def set_groups(groups):
    global GROUPS, GSTART, NG
    assert sum(groups) == IT
    GROUPS = list(groups)
    GSTART = [sum(GROUPS[:g]) for g in range(len(GROUPS))]
    NG = len(GROUPS)

_nc_cache: dict = {}


def _build(C: int, inv_sg: float, inv_su: float) -> bass.Bass:
    """One-core program; SPMD across 8 cores (same shapes, per-core data)."""
    nc = bass.Bass()
    GUW = 2 * KT * 128  # gate|up cols per i-tile (2048)
    DW = MT * 128  # down cols per i-tile (1024)
    xt = nc.dram_tensor("xt", [128, KT * C], _BF, kind="ExternalInput")
    if GU_FP8:
        w8t = nc.dram_tensor("w8t", [128, IT * GUW], _F8, kind="ExternalInput")
        wdt = nc.dram_tensor("wdt", [128, IT * DW], _BF, kind="ExternalInput")
    else:
        wt = nc.dram_tensor("wt", [128, IT * (GUW + DW)], _BF, kind="ExternalInput")
    yt = nc.dram_tensor("yt", [128, MT * C], _BF, kind="ExternalOutput")

    assert C + 256 <= 512, "two y slices must fit one PSUM bank"

    with ExitStack() as ctx:
        x_sb = ctx.enter_context(nc.sbuf_tensor([128, KT * C], _BF))
        if GU_FP8:
            w8_sb = ctx.enter_context(nc.sbuf_tensor([128, IT * GUW], _F8))
            wd_sb = ctx.enter_context(nc.sbuf_tensor([128, IT * DW], _BF))
        else:
            w_sb = ctx.enter_context(nc.sbuf_tensor([128, IT * (GUW + DW)], _BF))
        sg_sb = ctx.enter_context(nc.sbuf_tensor([128, IT * C], _F32))
        a_sb = ctx.enter_context(nc.sbuf_tensor([128, IT * C], _BF))
        # y writeback in bf16: halves the tail DMA and doubles copy rate
        # (costs ~0.2% extra output quantization, well inside the budget)
        y_sb = ctx.enter_context(nc.sbuf_tensor([128, MT * C], _BF))
        # every PSUM tensor is one full 2 KiB bank ([128, 512] f32): matmul
        # outputs must not cross bank boundaries, and the bump allocator
        # would otherwise pack tensors across banks
        g_ps = [
            ctx.enter_context(nc.psum_tensor(f"g_ps{j}", [128, 512], _F32))
            for j in range(2)
        ]
        u_ps = [
            ctx.enter_context(nc.psum_tensor(f"u_ps{j}", [128, 512], _F32))
            for j in range(2)
        ]
        y_ps = [
            ctx.enter_context(nc.psum_tensor(f"y_ps{j}", [128, 512], _F32))
            for j in range(4)
        ]

        def yslice(m):
            return y_ps[m // 2][:, (m % 2) * 256 : (m % 2) * 256 + C]

        def gw_tile(i, k):
            if GU_FP8:
                base = i * GUW
                return w8_sb[:, base + k * 128 : base + (k + 1) * 128]
            base = i * (GUW + DW)
            return w_sb[:, base + k * 128 : base + (k + 1) * 128]

        def uw_tile(i, k):
            if GU_FP8:
                base = i * GUW + KT * 128
                return w8_sb[:, base + k * 128 : base + (k + 1) * 128]
            base = i * (GUW + DW) + KT * 128
            return w_sb[:, base + k * 128 : base + (k + 1) * 128]

        def dw_tile(i, m):
            if GU_FP8:
                base = i * DW
                return wd_sb[:, base + m * 128 : base + (m + 1) * 128]
            base = i * (GUW + DW) + GUW
            return w_sb[:, base + m * 128 : base + (m + 1) * 128]

        warm_sb = ctx.enter_context(nc.sbuf_tensor([128, 512], _BF))

        x_sem = ctx.enter_context(nc.semaphore())
        x1_sem = ctx.enter_context(nc.semaphore(name="x1_sem"))
        x2_sem = ctx.enter_context(nc.semaphore(name="x2_sem"))
        warm_sem = ctx.enter_context(nc.semaphore(name="warm_sem"))
        w0_sem = [
            ctx.enter_context(nc.semaphore(name=f"w0_sem{j}")) for j in range(4)
        ]
        w_sem = [ctx.enter_context(nc.semaphore(name=f"w_sem{g}")) for g in range(NG)]
        if GU_FP8:
            wd_sem = [
                ctx.enter_context(nc.semaphore(name=f"wd_sem{g}")) for g in range(NG)
            ]
        pe_g = ctx.enter_context(nc.semaphore())
        pe_u = ctx.enter_context(nc.semaphore())
        pe_done = ctx.enter_context(nc.semaphore())
        act_sem = ctx.enter_context(nc.semaphore())
        dve_sem = ctx.enter_context(nc.semaphore())
        dma_sem = ctx.enter_context(nc.semaphore())

        # y writeback split: DVE copies m=0..4 (SP DMA), ACT copies m=5..7
        # (ACT's own HWDGE queue) — balances the two copy engines.
        MSPLIT = 5

        block = ctx.enter_context(nc.Block(no_gpsimd_drain=NO_GPSIMD_DRAIN))

        @block.sync
        def _(sync):
            # one SP HWDGE queue, chunks in exact PE-consumption order
            # (x split in two around the first gate/up chunk; down chunks
            # staggered one group behind gate/up since their first use is
            # one i-tile later). The queue's in-order descriptor stream
            # sustains ~410+ GB/s at 6KB/desc.
            def w8_dma(g):
                i0, gn = GSTART[g], GROUPS[g]
                nc.sync.dma_start(
                    w8_sb[:, i0 * GUW : (i0 + gn) * GUW],
                    w8t[:, i0 * GUW : (i0 + gn) * GUW],
                ).then_inc(w_sem[g], 16)

            def wd_dma(g):
                i0, gn = GSTART[g], GROUPS[g]
                nc.sync.dma_start(
                    wd_sb[:, i0 * DW : (i0 + gn) * DW],
                    wdt[:, i0 * DW : (i0 + gn) * DW],
                ).then_inc(wd_sem[g], 16)

            if GU_FP8:
                # x ships in three pieces (k0 | k1-3 | k4-7) and i-tile 0's
                # gate/up in four 64KB sub-chunks, interleaved so the PE's
                # first matmul needs only ~0.11MB landed.
                SUB = GUW // 4  # 512 cols = 4 k-tiles of one projection
                nc.sync.dma_start(x_sb[:, :C], xt[:, :C]).then_inc(x_sem, 16)
                nc.sync.dma_start(
                    w8_sb[:, :SUB], w8t[:, :SUB]
                ).then_inc(w0_sem[0], 16)
                nc.sync.dma_start(
                    x_sb[:, C : 4 * C], xt[:, C : 4 * C]
                ).then_inc(x1_sem, 16)
                nc.sync.dma_start(
                    x_sb[:, 4 * C :], xt[:, 4 * C :]
                ).then_inc(x2_sem, 16)
                for j in range(1, 4):
                    nc.sync.dma_start(
                        w8_sb[:, j * SUB : (j + 1) * SUB],
                        w8t[:, j * SUB : (j + 1) * SUB],
                    ).then_inc(w0_sem[j], 16)
                for g in range(1, NG):
                    w8_dma(g)
                    wd_dma(g - 1)
                wd_dma(NG - 1)
            else:
                nc.sync.dma_start(x_sb[:], xt[:]).then_inc(x_sem, 16)
                for g in range(NG):
                    i0, gn = GSTART[g], GROUPS[g]
                    W = GUW + DW
                    nc.sync.dma_start(
                        w_sb[:, i0 * W : (i0 + gn) * W],
                        wt[:, i0 * W : (i0 + gn) * W],
                    ).then_inc(w_sem[g], 16)
            nc.sync.wait_ge(dve_sem, IT + MSPLIT)
            nc.sync.dma_start(
                yt[:, : MSPLIT * C], y_sb[:, : MSPLIT * C]
            ).then_inc(dma_sem, 16)
            nc.sync.wait_ge(dma_sem, 32)

        def y_block(i, stop, inc_each=False):
            for m in range(MT):
                # start=True clears has_written for the WHOLE psum bank,
                # so only the first (even) slice of each bank may set it;
                # the odd slice's first write then lands on cleared
                # has_written and overwrites cleanly.
                mm = nc.tensor.matmul(
                    yslice(m),
                    dw_tile(i, m),
                    a_sb[:, i * C : (i + 1) * C],
                    start=(i == 0 and m % 2 == 0),
                    stop=stop,
                    skip_group_check=True,
                )
                if inc_each:
                    mm.then_inc(pe_done, 1)

        group_of = [g for g in range(NG) for _ in range(GROUPS[g])]

        @block.tensor
        def _(tensor):
            if GU_FP8 and WARMUP:
                # warm-up: garbage matmuls ramp the PE clock to full while
                # the first weight chunks stream in; psum bank 0 is
                # re-initialized (start=True) by the first real matmul.
                if SIM_WARMUP_WAIT:
                    nc.tensor.wait_ge(warm_sem, 1)
                for _ in range(WARMUP):
                    nc.tensor.matmul(
                        g_ps[0][:],
                        warm_sb[:, :128],
                        warm_sb[:],
                        start=True,
                        stop=True,
                    )
            nc.tensor.wait_ge(x_sem, 16)
            g = -1
            for i in range(IT):
                pp = i % 2
                if g + 1 < NG and i == GSTART[g + 1]:
                    g += 1
                    if not (GU_FP8 and g == 0):
                        nc.tensor.wait_ge(w_sem[g], 16)
                if i >= 2:
                    # covers g/u psum bank reuse (mul(i-2) drained) and,
                    # for Y_LAG==2, a_T[i-2] readiness for the y-block
                    nc.tensor.wait_ge(dve_sem, i - 1)
                for k in range(KT):
                    if GU_FP8 and i == 0:
                        if k == 0:
                            nc.tensor.wait_ge(w0_sem[0], 16)
                        elif k == 1:
                            nc.tensor.wait_ge(x1_sem, 16)
                        elif k == KT // 2:
                            nc.tensor.wait_ge(x2_sem, 16)
                            nc.tensor.wait_ge(w0_sem[1], 16)
                    mm = nc.tensor.matmul(
                        g_ps[pp][:, :C],
                        gw_tile(i, k),
                        x_sb[:, k * C : (k + 1) * C],
                        start=(k == 0),
                        stop=(k == KT - 1),
                    )
                mm.then_inc(pe_g, 1)
                for k in range(KT):
                    if GU_FP8 and i == 0 and k == 0:
                        nc.tensor.wait_ge(w0_sem[2], 16)
                    if GU_FP8 and i == 0 and k == KT // 2:
                        nc.tensor.wait_ge(w0_sem[3], 16)
                    mm = nc.tensor.matmul(
                        u_ps[pp][:, :C],
                        uw_tile(i, k),
                        x_sb[:, k * C : (k + 1) * C],
                        start=(k == 0),
                        stop=(k == KT - 1),
                    )
                mm.then_inc(pe_u, 1)
                if i >= Y_LAG:
                    # y-matmuls trail gate/up by Y_LAG i-tiles so ACT->DVE
                    # chain latency never stalls the PE
                    iy = i - Y_LAG
                    if Y_LAG == 1:
                        nc.tensor.wait_ge(dve_sem, i)
                    if GU_FP8 and iy in GSTART:
                        # down weights gate only their first y use
                        nc.tensor.wait_ge(wd_sem[group_of[iy]], 16)
                    y_block(iy, stop=False)
            for iy in range(IT - Y_LAG, IT - 1):
                nc.tensor.wait_ge(dve_sem, iy + 1)
                if GU_FP8 and iy in GSTART:
                    nc.tensor.wait_ge(wd_sem[group_of[iy]], 16)
                y_block(iy, stop=False)
            nc.tensor.wait_ge(dve_sem, IT)
            y_block(IT - 1, stop=True, inc_each=True)

        @block.scalar
        def _(scalar):
            for i in range(IT):
                pp = i % 2
                nc.scalar.wait_ge(pe_g, i + 1)
                nc.scalar.activation(
                    sg_sb[:, i * C : (i + 1) * C],
                    g_ps[pp][:, :C],
                    mybir.ActivationFunctionType.Silu,
                    scale=inv_sg,
                ).then_inc(act_sem, 1)
            for m in range(MSPLIT, MT):
                nc.scalar.wait_ge(pe_done, m + 1)
                nc.scalar.copy(y_sb[:, m * C : (m + 1) * C], yslice(m)).then_inc(
                    act_sem, 1
                )
            nc.scalar.wait_ge(act_sem, IT + (MT - MSPLIT))
            nc.scalar.dma_start(
                yt[:, MSPLIT * C :], y_sb[:, MSPLIT * C :]
            ).then_inc(dma_sem, 16)

        @block.vector
        def _(vector):
            if GU_FP8 and SIM_WARMUP_WAIT:
                nc.vector.memset(warm_sb[:], 0.0).then_inc(warm_sem, 1)
            for i in range(IT):
                pp = i % 2
                nc.vector.wait_ge(act_sem, i + 1)
                nc.vector.wait_ge(pe_u, i + 1)
                if GU_FP8:
                    # a = (u' * 1/Su) * silu_g   (1/Su is a power of two)
                    nc.vector.scalar_tensor_tensor(
                        a_sb[:, i * C : (i + 1) * C],
                        u_ps[pp][:, :C],
                        inv_su,
                        sg_sb[:, i * C : (i + 1) * C],
                        AluOpType.mult,
                        AluOpType.mult,
                    ).then_inc(dve_sem, 1)
                else:
                    nc.vector.tensor_mul(
                        a_sb[:, i * C : (i + 1) * C],
                        sg_sb[:, i * C : (i + 1) * C],
                        u_ps[pp][:, :C],
                    ).then_inc(dve_sem, 1)
            for m in range(MSPLIT):
                nc.vector.wait_ge(pe_done, m + 1)
                nc.vector.tensor_copy(
                    y_sb[:, m * C : (m + 1) * C], yslice(m)
                ).then_inc(dve_sem, 1)

    return nc


def _bf(x):
    return np.ascontiguousarray(x).astype(ml_dtypes.bfloat16)


def _pow2_scale(absmax: float, dt) -> float:
    fmax = float(ml_dtypes.finfo(dt).max)
    return 2.0 ** math.floor(math.log2((fmax * 0.5) / absmax))


def run(hidden_states, router_w, gate_w, up_w, down_w, trace=False):
    h = np.asarray(hidden_states, dtype=np.float32)
    rw = np.asarray(router_w, dtype=np.float32)
    gw = np.asarray(gate_w, dtype=np.float32)
    uw = np.asarray(up_w, dtype=np.float32)
    dw = np.asarray(down_w, dtype=np.float32)

    T = S * B
    hf = h.reshape(T, H)
    logits = hf.astype(np.float64) @ rw.astype(np.float64).T
    ids = logits.argmax(-1)
    idx = [np.where(ids == e)[0] for e in range(E)]
    maxc = max(len(s) for s in idx)
    C = max(128, -(-maxc // 4) * 4)

    if GU_FP8:
        sg = _pow2_scale(float(np.abs(gw).max()), ml_dtypes.float8_e3m4)
        su = _pow2_scale(float(np.abs(uw).max()), ml_dtypes.float8_e3m4)
    else:
        sg = su = 1.0

    key = (C, sg, su, Y_LAG, WARMUP, NO_GPSIMD_DRAIN, GROUPS_KEY())
    if key not in _nc_cache:
        _nc_cache[key] = _build(C, 1.0 / sg, 1.0 / su)
    nc = _nc_cache[key]

    in_maps = []
    for e in range(E):
        sel = idx[e]
        xp = np.zeros((C, H), np.float32)
        xp[: len(sel)] = hf[sel]
        # xt[p, k*C+c] = x[c, k*128+p]
        xt = _bf(xp.reshape(C, KT, 128).transpose(2, 1, 0).reshape(128, KT * C))
        # gwt[i, p, k*128+m] = gate_w[e][i*128+m, k*128+p]
        gwt = gw[e].reshape(IT, 128, KT, 128).transpose(0, 3, 2, 1).reshape(IT, 128, KT * 128)
        uwt = uw[e].reshape(IT, 128, KT, 128).transpose(0, 3, 2, 1).reshape(IT, 128, KT * 128)
        # dwt[i, p, m*128+mm] = down_w[e][m*128+mm, i*128+p]
        dwt = dw[e].reshape(MT, 128, IT, 128).transpose(2, 3, 0, 1).reshape(IT, 128, MT * 128)
        if GU_FP8:
            gu = np.concatenate([gwt * sg, uwt * su], axis=2)  # [IT,128,2048]
            w8 = np.ascontiguousarray(
                gu.transpose(1, 0, 2).reshape(128, IT * 2 * KT * 128)
            ).astype(ml_dtypes.float8_e3m4)
            wdv = _bf(dwt.transpose(1, 0, 2).reshape(128, IT * MT * 128))
            in_maps.append({"xt": xt, "w8t": w8, "wdt": wdv})
        else:
            wtv = _bf(
                np.concatenate([gwt, uwt, dwt], axis=2)
                .transpose(1, 0, 2)
                .reshape(128, IT * (2 * KT + MT) * 128)
            )
            in_maps.append({"xt": xt, "wt": wtv})

    res = run_bass_kernel_spmd(nc, in_maps, core_ids=list(range(E)), trace=trace)

    out = np.zeros((T, H), np.float32)
    for e in range(E):
        ytv = np.asarray(res.results[e]["yt"]).astype(np.float32)
        # y[c, m*128+p] = yt[p, m*C+c]
        y = ytv.reshape(128, MT, C).transpose(2, 1, 0).reshape(C, H)
        out[idx[e]] = y[: len(idx[e])]
    return out.reshape(S, B, H), res


def kernel(**inputs) -> np.ndarray:
    out, _ = run(**inputs)
    return out
